# revision 16
# baseline (speedup 1.0000x reference)
"""Trainium2 Bass kernel for nn_DecoderRNN (5 chained LSTM cells + shared linear
head + softmax), batch=1, tensor-parallel over 8 NeuronCores.

Sharding (per core c of 8):
  * Each LSTM's gate rows (4H = 8192) are sharded 1024/core, interleaved so that
    core c owns gate rows r with (r mod 128) in [16c, 16c+16). Each core computes
    its (1, 1024) slice of the gate pre-activations (Wih@x + Whh@h + b) and an
    AllGather yields the full (8192,) vector on every core, laid out so a single
    contiguous DMA loads it as a (128, 64) SBUF tile in "column-major" vector
    layout ([p, j] = gates[j*128 + p]).
  * The elementwise LSTM update (sigmoid/tanh gates, c/h update) is computed
    redundantly on every core (tiny).
  * The shared head W_out (1024, 2048) is sharded by output row: core c computes
    y[c*128:(c+1)*128]; a second AllGather rebuilds full y for the next layer.
  * Weights stream through SBUF as bf16 hi/lo pairs (same total bytes as fp32)
    and each mat-vec runs as 3 bf16 matmul passes (hi*hi + hi*lo + lo*hi),
    accumulating in fp32 PSUM: ~1e-5 relative error at full PE streaming rate.
    The small head matmul runs in exact fp32.

All per-core inputs are packed into two DRAM blobs (bf16 weights / fp32
vectors) and all outputs into one fp32 blob — the PJRT execute path pays a
large fixed cost per bound buffer, and 3 buffers instead of 47 keeps the
dispatch out of the measurement (and off the critical path of any caller).

kernel(**inputs) takes the FULL unsharded inputs (same keys as
reference.setup_inputs()), does all sharding/layout prep host-side in numpy,
runs the SPMD Bass program on cores 0-7, and reassembles the full outputs.
"""

import numpy as np
import ml_dtypes

import concourse.bass as bass
import concourse.bacc as bacc
import concourse.tile as tile
import concourse.mybir as mybir

H = 2048
O = 1024
NC = 8
F32 = mybir.dt.float32
BF16 = mybir.dt.bfloat16
AF = mybir.ActivationFunctionType

# wblob row layout: per layer k, 1024 rows of WihT pack then 2048 rows of
# WhhT pack; every row is 2048 bf16 ([hi(1024) | lo(1024)]).
WROWS_PER_LAYER = O + H               # 3072
WBLOB_ROWS = 5 * WROWS_PER_LAYER      # 15360


def _vblob_layout():
    off = {}
    cur = 0
    for name, n in [("x1cm", O), ("ident8", 64), ("one", 1),
                    ("ones2d", 128 * 128), ("boutc", 128),
                    ("woutts", H * 128)] + \
                   [(f"b{k}", O) for k in range(1, 6)] + \
                   [(f"hcm{k}", H) for k in range(1, 6)] + \
                   [(f"ccm{k}", H) for k in range(1, 6)]:
        off[name] = (cur, n)
        cur += (n + 511) // 512 * 512
    return off, cur


VOFF, VLEN = _vblob_layout()

OOFF = {"outp": (0, O)}
_cur = O
for _k in range(1, 6):
    OOFF[f"hn{_k}"] = (_cur, H); _cur += H
    OOFF[f"cn{_k}"] = (_cur, H); _cur += H
OLEN = _cur


# --------------------------------------------------------------------------
# Device program
# --------------------------------------------------------------------------

def build_nc(ablate_gate_mms=False, ablate_weight_dma=False, local_ag=False,
             num_devices=NC, wbufs=36, lookahead=2, split_slab=2,
             small_on_scalar=False, two_pass=False):
    nc = bacc.Bacc("TRN2", target_bir_lowering=False, debug=False,
                   num_devices=num_devices)
    wblob = nc.dram_tensor("wblob", [WBLOB_ROWS, 2 * O], BF16,
                           kind="ExternalInput").ap()
    vblob = nc.dram_tensor("vblob", [VLEN], F32, kind="ExternalInput").ap()
    oblob = nc.dram_tensor("oblob", [OLEN], F32, kind="ExternalOutput").ap()

    def vsl(name):
        off, n = VOFF[name]
        return vblob[off:off + n]

    def osl(name):
        off, n = OOFF[name]
        return oblob[off:off + n]

    with tile.TileContext(nc) as tc:
        with (
            tc.tile_pool(name="wpool", bufs=wbufs) as wpool,
            tc.tile_pool(name="small", bufs=1) as small,
            tc.tile_pool(name="work", bufs=2) as work,
            tc.tile_pool(name="psg", bufs=3, space="PSUM") as psg,
            tc.tile_pool(name="psy", bufs=1, space="PSUM") as psy,
            tc.tile_pool(name="pst", bufs=1, space="PSUM") as pst,
            tc.tile_pool(name="dpool", bufs=1, space="DRAM") as dpool,
        ):
            # ---------- phase A: small input loads (sync engine) ----------
            def load(name, shape, in_ap):
                t = small.tile(shape, F32, name=f"sb_{name}")
                eng = nc.scalar if small_on_scalar else nc.sync
                eng.dma_start(t[:], in_ap)
                return t

            x1_sb = load("x1cm", [128, 8], vsl("x1cm").rearrange("(m p) -> p m", m=8))
            one_sb = load("one", [1, 1], vsl("one").rearrange("(a b) -> a b", a=1))
            ones2d_sb = load("ones2d", [128, 128],
                             vsl("ones2d").rearrange("(p j) -> p j", p=128))
            id8_sb = load("ident8", [8, 8], vsl("ident8").rearrange("(a b) -> a b", a=8))
            boutc_sb = load("boutc", [128, 1], vsl("boutc").rearrange("(p a) -> p a", p=128))
            h_sb = {k: load(f"hcm{k}", [128, 16],
                            vsl(f"hcm{k}").rearrange("(j p) -> p j", j=16))
                    for k in range(1, 6)}
            c_sb = {k: load(f"ccm{k}", [128, 16],
                            vsl(f"ccm{k}").rearrange("(j p) -> p j", j=16))
                    for k in range(1, 6)}
            b_sb = {k: load(f"b{k}", [1, O], vsl(f"b{k}").rearrange("(a n) -> a n", a=1))
                    for k in range(1, 6)}
            wout_sb = small.tile([128, 16 * 128], F32, name="wout_sb")
            (nc.scalar if small_on_scalar else nc.sync).dma_start(
                wout_sb[:].rearrange("p (t m) -> p t m", t=16),
                vsl("woutts").rearrange("(t p m) -> p t m", t=16, p=128))

            # ---------- phase B: weight slab DMAs in PE consumption order ----------
            wih_slabs = {k: [None] * 8 for k in range(1, 6)}
            whh_slabs = {k: [None] * 16 for k in range(1, 6)}

            def emit_whh_dma(k):
                if ablate_weight_dma:
                    return
                r0 = (k - 1) * WROWS_PER_LAYER + O
                for t in range(16):
                    s = wpool.tile([128, 2 * O], BF16, name=f"whh{k}_{t}",
                                   tag="wslab")
                    if split_slab:
                        q = 2 * O // split_slab
                        for si in range(split_slab):
                            nc.sync.dma_start(
                                s[:, si * q:(si + 1) * q],
                                wblob[r0 + t * 128:r0 + (t + 1) * 128, si * q:(si + 1) * q])
                    else:
                        nc.sync.dma_start(s[:], wblob[r0 + t * 128:r0 + (t + 1) * 128, :])
                    whh_slabs[k][t] = s

            def emit_wih_dma(k):
                if ablate_weight_dma:
                    return
                r0 = (k - 1) * WROWS_PER_LAYER
                for t in range(8):
                    s = wpool.tile([128, 2 * O], BF16, name=f"wih{k}_{t}",
                                   tag="wslab")
                    if split_slab:
                        q = 2 * O // split_slab
                        for si in range(split_slab):
                            nc.sync.dma_start(
                                s[:, si * q:(si + 1) * q],
                                wblob[r0 + t * 128:r0 + (t + 1) * 128, si * q:(si + 1) * q])
                    else:
                        nc.sync.dma_start(s[:], wblob[r0 + t * 128:r0 + (t + 1) * 128, :])
                    wih_slabs[k][t] = s

            # consumption order: w1 i1 w2 w3 i2 w4 i3 w5 i4 i5
            emit_whh_dma(1); emit_wih_dma(1)
            emit_whh_dma(2); emit_whh_dma(3)
            emit_wih_dma(2)
            emit_whh_dma(4); emit_wih_dma(3)
            emit_whh_dma(5); emit_wih_dma(4)
            emit_wih_dma(5)

            # ---------- phase C: hi/lo splits of stationary vectors (DVE) ----------
            def split(src, F, nm):
                hi = small.tile([128, F], BF16, name=f"{nm}_hi")
                nc.vector.tensor_copy(hi[:], src[:])
                hi32 = small.tile([128, F], F32, name=f"{nm}_hi32")
                nc.vector.tensor_copy(hi32[:], hi[:])
                res = small.tile([128, F], F32, name=f"{nm}_res")
                nc.vector.tensor_sub(res[:], src[:], hi32[:])
                lo = small.tile([128, F], BF16, name=f"{nm}_lo")
                nc.vector.tensor_copy(lo[:], res[:])
                return hi, lo

            if two_pass:
                hsplit = {}
                xsplit = {}
                xstat = {1: x1_sb}
            else:
                hsplit = {k: split(h_sb[k], 16, f"h{k}") for k in range(1, 6)}
                xsplit = {1: split(x1_sb, 8, "x1")}
                xstat = {}

            # ---------- phase D: layered compute ----------
            psum_g = {}
            agin = {k: dpool.tile([1, O], F32, name=f"agin{k}") for k in range(1, 6)}
            agout = {k: dpool.tile([128, 64], F32, name=f"agout{k}") for k in range(1, 6)}
            ag2in = {k: dpool.tile([128, 1], F32, name=f"ag2in{k}") for k in range(1, 6)}
            ag2out = {k: dpool.tile([8, 128], F32, name=f"ag2out{k}") for k in range(1, 6)}
            y_cm = {}
            h_new = {}
            replica = [list(range(num_devices))]

            def emit_gates_whh(k):
                """bias + Whh@h part of layer k's gate pre-activations (PE)."""
                pg = psg.tile([1, O], F32, name=f"psg{k}", tag="psg")
                psum_g[k] = pg
                bias_stop = ablate_gate_mms or ablate_weight_dma
                for n0 in (0, 512):
                    nc.tensor.matmul(pg[0:1, n0:n0 + 512], one_sb[:],
                                     b_sb[k][0:1, n0:n0 + 512],
                                     start=True, stop=bias_stop)
                if ablate_gate_mms or ablate_weight_dma:
                    return
                if two_pass:
                    passes = ((h_sb[k], 0), (h_sb[k], O))
                else:
                    hh, hl = hsplit[k]
                    passes = ((hh, 0), (hh, O), (hl, 0))
                for t in range(16):
                    s = whh_slabs[k][t]
                    for lh, base in passes:
                        for n0 in (0, 512):
                            nc.tensor.matmul(pg[0:1, n0:n0 + 512],
                                             lh[:, t:t + 1],
                                             s[:, base + n0:base + n0 + 512],
                                             start=False, stop=False)

            def emit_gates_wih(k):
                """Wih@x part (PE) + psum->DRAM + AllGather of gate slice."""
                pg = psum_g[k]
                if two_pass:
                    xs = xstat[k]
                    passes = ((xs, 0), (xs, O))
                else:
                    xh, xl = xsplit[k]
                    passes = ((xh, 0), (xh, O), (xl, 0))
                if not (ablate_gate_mms or ablate_weight_dma):
                    for t in range(8):
                        s = wih_slabs[k][t]
                        for pi, (lh, base) in enumerate(passes):
                            last = (t == 7 and pi == len(passes) - 1)
                            for n0 in (0, 512):
                                nc.tensor.matmul(pg[0:1, n0:n0 + 512],
                                                 lh[:, t:t + 1],
                                                 s[:, base + n0:base + n0 + 512],
                                                 start=False, stop=last)
                gsb = work.tile([1, O], F32, name=f"gsb{k}", tag="gsb")
                nc.vector.tensor_copy(gsb[:], pg[:])
                nc.scalar.dma_start(agin[k][:], gsb[:])
                if local_ag:
                    nc.scalar.dma_start(
                        agout[k].rearrange("p j -> (p j)")[0:O], agin[k][0, :])
                else:
                    nc.gpsimd.collective_compute(
                        "AllGather", mybir.AluOpType.bypass,
                        replica_groups=replica,
                        ins=[agin[k].opt()], outs=[agout[k].opt()])

            def emit_elem(k):
                """Gate nonlinearities + c/h update (ACT + DVE), store hn/cn."""
                gates = work.tile([128, 64], F32, name=f"gates{k}", tag="gates")
                nc.scalar.dma_start(gates[:], agout[k][:])
                si = work.tile([128, 16], F32, name=f"si{k}", tag="si")
                sf = work.tile([128, 16], F32, name=f"sf{k}", tag="sf")
                tg = work.tile([128, 16], F32, name=f"tg{k}", tag="tg")
                so = work.tile([128, 16], F32, name=f"so{k}", tag="so")
                nc.scalar.activation(si[:], gates[:, 0:16], AF.Sigmoid)
                nc.scalar.activation(sf[:], gates[:, 16:32], AF.Sigmoid)
                nc.scalar.activation(tg[:], gates[:, 32:48], AF.Tanh)
                nc.scalar.activation(so[:], gates[:, 48:64], AF.Sigmoid)
                t1 = work.tile([128, 16], F32, name=f"t1_{k}", tag="t1")
                t2 = work.tile([128, 16], F32, name=f"t2_{k}", tag="t2")
                cn = work.tile([128, 16], F32, name=f"cn{k}", tag="cnew")
                tcn = work.tile([128, 16], F32, name=f"tc{k}", tag="tcn")
                hn = work.tile([128, 16], F32, name=f"hn{k}", tag="hnew")
                nc.vector.tensor_mul(t1[:], sf[:], c_sb[k][:])
                nc.vector.tensor_mul(t2[:], si[:], tg[:])
                nc.vector.tensor_add(cn[:], t1[:], t2[:])
                nc.scalar.activation(tcn[:], cn[:], AF.Tanh)
                nc.vector.tensor_mul(hn[:], so[:], tcn[:])
                h_new[k] = hn
                nc.scalar.dma_start(
                    osl(f"hn{k}").rearrange("(p j) -> p j", p=128), hn[:])
                nc.scalar.dma_start(
                    osl(f"cn{k}").rearrange("(p j) -> p j", p=128), cn[:])

            def emit_head(k):
                """Sharded head: y[c*128:(c+1)*128] = Wout_slice @ hn + bout_slice."""
                py = psy.tile([128, 1], F32, name=f"psy{k}", tag="psy")
                for t in range(16):
                    nc.tensor.matmul(py[:], wout_sb[:, t * 128:(t + 1) * 128],
                                     h_new[k][:, t:t + 1],
                                     start=(t == 0), stop=(t == 15))
                ysb = work.tile([128, 1], F32, name=f"ysb{k}", tag="ysb")
                nc.vector.tensor_add(ysb[:], py[:], boutc_sb[:])
                nc.scalar.dma_start(ag2in[k][:], ysb[:])
                if local_ag:
                    nc.scalar.dma_start(
                        ag2out[k].rearrange("m p -> (m p)")[0:128],
                        ag2in[k][:, 0])
                else:
                    nc.gpsimd.collective_compute(
                        "AllGather", mybir.AluOpType.bypass,
                        replica_groups=replica,
                        ins=[ag2in[k].opt()], outs=[ag2out[k].opt()])

            def emit_redist(k):
                """Full y back to column-major (128, 8) via PE transpose."""
                yrow = work.tile([8, 128], F32, name=f"yrow{k}", tag="yrow")
                nc.scalar.dma_start(yrow[:], ag2out[k][:])
                pt = pst.tile([128, 8], F32, name=f"pst{k}", tag="pst")
                nc.tensor.transpose(pt[:], yrow[:], id8_sb[:])
                yc = small.tile([128, 8], F32, name=f"ycm{k}")
                nc.vector.tensor_copy(yc[:], pt[:])
                y_cm[k] = yc
                if k < 5:
                    if k == 1:
                        xn = yc
                    else:
                        xn = small.tile([128, 8], F32, name=f"x{k+1}cm")
                        nc.vector.tensor_add(xn[:], yc[:], y_cm[k - 1][:])
                    if two_pass:
                        xstat[k + 1] = xn
                    else:
                        xsplit[k + 1] = split(xn, 8, f"x{k+1}")

            def emit_softmax():
                e = small.tile([128, 8], F32, name="esm")
                nc.scalar.activation(e[:], y_cm[5][:], AF.Exp)
                s = small.tile([128, 1], F32, name="ssm")
                nc.vector.reduce_sum(s[:], e[:], axis=mybir.AxisListType.X)
                tot = psy.tile([128, 1], F32, name="totps", tag="psy")
                nc.tensor.matmul(tot[:], ones2d_sb[:], s[:], start=True, stop=True)
                inv = small.tile([128, 1], F32, name="invsm")
                nc.vector.reciprocal(inv[:], tot[:])
                osb = small.tile([128, 8], F32, name="osb")
                nc.vector.tensor_scalar_mul(osb[:], e[:], inv[:])
                nc.scalar.dma_start(
                    osl("outp").rearrange("(p m) -> p m", p=128), osb[:])

            # Whh lookahead keeps PE fed during collective waits
            emit_gates_whh(1)
            emit_gates_wih(1)
            emitted_whh = 1
            for k in range(1, 6):
                while emitted_whh < min(5, k + lookahead - 1):
                    emitted_whh += 1
                    emit_gates_whh(emitted_whh)
                emit_elem(k)
                emit_head(k)
                if emitted_whh < min(5, k + lookahead):
                    emitted_whh += 1
                    emit_gates_whh(emitted_whh)
                emit_redist(k)
                if k + 1 <= 5:
                    emit_gates_wih(k + 1)
            emit_softmax()

    nc.compile()
    return nc


# --------------------------------------------------------------------------
# Host-side input prep (sharding + layout + blob packing)
# --------------------------------------------------------------------------

def prep_in_maps(inputs):
    inp = {k: np.asarray(v, dtype=np.float32) for k, v in inputs.items()}
    x1 = inp["input"].reshape(O)
    WoutT = np.ascontiguousarray(inp["Wout"].T)          # (H, O)
    bout = inp["bout"].reshape(O)

    def pack(wt):
        hi = wt.astype(ml_dtypes.bfloat16)
        lo = (wt - hi.astype(np.float32)).astype(ml_dtypes.bfloat16)
        return np.concatenate([hi, lo], axis=1)

    in_maps = []
    for c in range(NC):
        pp, jj = np.meshgrid(np.arange(16), np.arange(64), indexing="ij")
        idx = (jj * 128 + 16 * c + pp).reshape(-1)       # slice row s -> gate row

        wrows = []
        for k in range(1, 6):
            wrows.append(pack(inp[f"Wih{k}"][idx, :].T))     # (O, 2O)
            wrows.append(pack(inp[f"Whh{k}"][idx, :].T))     # (H, 2O)
        wblob = np.ascontiguousarray(np.concatenate(wrows, axis=0))

        v = np.zeros(VLEN, np.float32)

        def put(name, arr):
            off, n = VOFF[name]
            a = np.asarray(arr, np.float32).ravel()
            assert a.size == n, (name, a.size, n)
            v[off:off + n] = a

        put("x1cm", x1)                       # flat natural; device rearranges
        put("ident8", np.eye(8, dtype=np.float32))
        put("one", [1.0])
        put("ones2d", np.ones(128 * 128, np.float32))
        put("boutc", bout[c * 128:(c + 1) * 128])
        put("woutts", WoutT[:, c * 128:(c + 1) * 128].reshape(16, 128, 128))
        for k in range(1, 6):
            put(f"b{k}", (inp[f"bih{k}"] + inp[f"bhh{k}"])[idx])
            put(f"hcm{k}", inp[f"h{k}"].reshape(H))
            put(f"ccm{k}", inp[f"c{k}"].reshape(H))
        in_maps.append({"wblob": wblob, "vblob": v})
    return in_maps


# --------------------------------------------------------------------------
# Cached PJRT runner (the axon redirect path of bass_utils.run_bass_kernel_spmd,
# with the jitted executable cached so repeated kernel() calls don't recompile)
# --------------------------------------------------------------------------

_RT = None


def _make_runner(nc):
    import jax
    from jax.sharding import Mesh, PartitionSpec
    from jax.experimental.shard_map import shard_map
    from concourse import bass2jax
    bass2jax.install_neuronx_cc_hook()

    partition_name = (nc.partition_id_tensor.name
                      if nc.partition_id_tensor is not None else None)
    in_names, out_names, out_avals, zero_shapes = [], [], [], []
    for alloc in nc.m.functions[0].allocations:
        if not isinstance(alloc, mybir.MemoryLocationSet):
            continue
        name = alloc.memorylocations[0].name
        if alloc.kind == "ExternalInput":
            if name != partition_name:
                in_names.append(name)
        elif alloc.kind == "ExternalOutput":
            out_names.append(name)
            shape = tuple(alloc.tensor_shape)
            dtype = mybir.dt.np(alloc.dtype)
            out_avals.append(jax.core.ShapedArray(shape, dtype))
            zero_shapes.append((shape, dtype))
    n_params = len(in_names)
    n_outs = len(out_names)
    all_in_names = tuple(in_names) + tuple(out_names)
    if partition_name is not None:
        all_in_names = all_in_names + (partition_name,)

    def _body(*args):
        operands = list(args)
        if partition_name is not None:
            operands.append(bass2jax.partition_id_tensor())
        outs = bass2jax._bass_exec_p.bind(
            *operands,
            out_avals=tuple(out_avals),
            in_names=all_in_names,
            out_names=tuple(out_names),
            lowering_input_output_aliases=(),
            sim_require_finite=True,
            sim_require_nnan=True,
            nc=nc,
        )
        return tuple(outs)

    devices = jax.devices()[:NC]
    mesh = Mesh(np.asarray(devices), ("core",))
    sharded = jax.jit(
        shard_map(_body, mesh=mesh,
                  in_specs=(PartitionSpec("core"),) * (n_params + n_outs),
                  out_specs=(PartitionSpec("core"),) * n_outs,
                  check_rep=False),
        donate_argnums=tuple(range(n_params, n_params + n_outs)),
        keep_unused=True,
    )
    return sharded, in_names, out_names, out_avals, zero_shapes


def get_runtime():
    global _RT
    if _RT is None:
        nc = build_nc()
        _RT = _make_runner(nc)
    return _RT


def run_in_maps(in_maps):
    """Run the SPMD program; returns core 0's output map."""
    sharded, in_names, out_names, out_avals, zero_shapes = get_runtime()
    concat_in = [np.concatenate([in_maps[c][nm] for c in range(NC)], axis=0)
                 for nm in in_names]
    concat_zeros = [np.zeros((NC * s[0], *s[1:]), dt) for s, dt in zero_shapes]
    outs = sharded(*concat_in, *concat_zeros)
    return {nm: np.asarray(outs[i]).reshape(NC, *out_avals[i].shape)[0]
            for i, nm in enumerate(out_names)}


# --------------------------------------------------------------------------
# Public entry point
# --------------------------------------------------------------------------

def kernel(**inputs):
    res0 = run_in_maps(prep_in_maps(inputs))
    ob = res0["oblob"]

    def get(name, j):
        # stored [p, j] row-major; natural vector order is j*128+p
        off, n = OOFF[name]
        return np.ascontiguousarray(ob[off:off + n].reshape(128, j).T).ravel()

    out = get("outp", 8).reshape(1, O).astype(np.float32)
    ret = [out]
    for k in range(1, 6):
        hn = get(f"hn{k}", 16).reshape(1, 1, H).astype(np.float32)
        cn = get(f"cn{k}", 16).reshape(1, 1, H).astype(np.float32)
        ret += [hn, cn]
    return tuple(ret)


# revision 21
# speedup vs baseline: 1.0309x; 1.0309x over previous
"""Trainium2 Bass kernel for nn_DecoderRNN (5 chained LSTM cells + shared linear
head + softmax), batch=1, tensor-parallel over 8 NeuronCores.

Sharding (per core c of 8):
  * Each LSTM's gate rows (4H = 8192) are sharded 1024/core, interleaved so that
    core c owns gate rows r with (r mod 128) in [16c, 16c+16). Each core computes
    its (1, 1024) slice of the gate pre-activations (Wih@x + Whh@h + b) and an
    AllGather yields the full (8192,) vector on every core, laid out so a single
    contiguous DMA loads it as a (128, 64) SBUF tile in "column-major" vector
    layout ([p, j] = gates[j*128 + p]).
  * The elementwise LSTM update (sigmoid/tanh gates, c/h update) is computed
    redundantly on every core (tiny).
  * The shared head W_out (1024, 2048) is sharded by output row: core c computes
    y[c*128:(c+1)*128]; a second AllGather rebuilds full y for the next layer.
  * Weights stream through SBUF as bf16 hi/lo pairs (same total bytes as fp32)
    and each mat-vec runs as 3 bf16 matmul passes (hi*hi + hi*lo + lo*hi),
    accumulating in fp32 PSUM: ~1e-5 relative error at full PE streaming rate.
    The small head matmul runs in exact fp32.

All per-core inputs are packed into two DRAM blobs (bf16 weights / fp32
vectors) and all outputs into one fp32 blob — the PJRT execute path pays a
large fixed cost per bound buffer, and 3 buffers instead of 47 keeps the
dispatch out of the measurement (and off the critical path of any caller).

kernel(**inputs) takes the FULL unsharded inputs (same keys as
reference.setup_inputs()), does all sharding/layout prep host-side in numpy,
runs the SPMD Bass program on cores 0-7, and reassembles the full outputs.
"""

import numpy as np
import ml_dtypes

import concourse.bass as bass
import concourse.bacc as bacc
import concourse.tile as tile
import concourse.mybir as mybir

H = 2048
O = 1024
NC = 8
F32 = mybir.dt.float32
BF16 = mybir.dt.bfloat16
AF = mybir.ActivationFunctionType

# wblob row layout: per layer k, 1024 rows of WihT pack then 2048 rows of
# WhhT pack; every row is 2048 bf16 ([hi(1024) | lo(1024)]).
WROWS_PER_LAYER = O + H               # 3072
WBLOB_ROWS = 5 * WROWS_PER_LAYER      # 15360


def _vblob_layout():
    off = {}
    cur = 0
    for name, n in [("x1cm", O), ("ident8", 64), ("one", 1),
                    ("ones2d", 128 * 128), ("boutc", 128),
                    ("woutts", H * 128)] + \
                   [(f"b{k}", O) for k in range(1, 6)] + \
                   [(f"hcm{k}", H) for k in range(1, 6)] + \
                   [(f"ccm{k}", H) for k in range(1, 6)]:
        off[name] = (cur, n)
        cur += (n + 511) // 512 * 512
    return off, cur


VOFF, VLEN = _vblob_layout()

OOFF = {"outp": (0, O)}
_cur = O
for _k in range(1, 6):
    OOFF[f"hn{_k}"] = (_cur, H); _cur += H
    OOFF[f"cn{_k}"] = (_cur, H); _cur += H
OLEN = _cur


# --------------------------------------------------------------------------
# Device program
# --------------------------------------------------------------------------

def build_nc(ablate_gate_mms=False, ablate_weight_dma=False, local_ag=False,
             num_devices=NC, wbufs=36, lookahead=2, split_slab=2,
             small_on_scalar=False, two_pass=False):
    nc = bacc.Bacc("TRN2", target_bir_lowering=False, debug=False,
                   num_devices=num_devices)
    wblob = nc.dram_tensor("wblob", [WBLOB_ROWS, 2 * O], BF16,
                           kind="ExternalInput").ap()
    vblob = nc.dram_tensor("vblob", [VLEN], F32, kind="ExternalInput").ap()
    oblob = nc.dram_tensor("oblob", [OLEN], F32, kind="ExternalOutput").ap()

    def vsl(name):
        off, n = VOFF[name]
        return vblob[off:off + n]

    def osl(name):
        off, n = OOFF[name]
        return oblob[off:off + n]

    with tile.TileContext(nc) as tc:
        with (
            tc.tile_pool(name="wpool", bufs=wbufs) as wpool,
            tc.tile_pool(name="small", bufs=1) as small,
            tc.tile_pool(name="work", bufs=2) as work,
            tc.tile_pool(name="psg", bufs=3, space="PSUM") as psg,
            tc.tile_pool(name="psy", bufs=1, space="PSUM") as psy,
            tc.tile_pool(name="pst", bufs=1, space="PSUM") as pst,
            tc.tile_pool(name="dpool", bufs=1, space="DRAM") as dpool,
        ):
            # ---------- phase A: small input loads (sync engine) ----------
            def load(name, shape, in_ap):
                t = small.tile(shape, F32, name=f"sb_{name}")
                eng = nc.scalar if small_on_scalar else nc.sync
                eng.dma_start(t[:], in_ap)
                return t

            x1_sb = load("x1cm", [128, 8], vsl("x1cm").rearrange("(m p) -> p m", m=8))
            one_sb = load("one", [1, 1], vsl("one").rearrange("(a b) -> a b", a=1))
            ones2d_sb = load("ones2d", [128, 128],
                             vsl("ones2d").rearrange("(p j) -> p j", p=128))
            id8_sb = load("ident8", [8, 8], vsl("ident8").rearrange("(a b) -> a b", a=8))
            boutc_sb = load("boutc", [128, 1], vsl("boutc").rearrange("(p a) -> p a", p=128))
            h_sb = {k: load(f"hcm{k}", [128, 16],
                            vsl(f"hcm{k}").rearrange("(j p) -> p j", j=16))
                    for k in range(1, 6)}
            c_sb = {k: load(f"ccm{k}", [128, 16],
                            vsl(f"ccm{k}").rearrange("(j p) -> p j", j=16))
                    for k in range(1, 6)}
            b_sb = {k: load(f"b{k}", [1, O], vsl(f"b{k}").rearrange("(a n) -> a n", a=1))
                    for k in range(1, 6)}
            wout_sb = small.tile([128, 16 * 128], F32, name="wout_sb")
            (nc.scalar if small_on_scalar else nc.sync).dma_start(
                wout_sb[:].rearrange("p (t m) -> p t m", t=16),
                vsl("woutts").rearrange("(t p m) -> p t m", t=16, p=128))

            # ---------- phase B: weight slab DMAs in PE consumption order ----------
            wih_slabs = {k: [None] * 8 for k in range(1, 6)}
            whh_slabs = {k: [None] * 16 for k in range(1, 6)}

            def emit_whh_dma(k):
                if ablate_weight_dma:
                    return
                r0 = (k - 1) * WROWS_PER_LAYER + O
                for t in range(16):
                    s = wpool.tile([128, 2 * O], BF16, name=f"whh{k}_{t}",
                                   tag="wslab")
                    if split_slab:
                        q = 2 * O // split_slab
                        for si in range(split_slab):
                            nc.sync.dma_start(
                                s[:, si * q:(si + 1) * q],
                                wblob[r0 + t * 128:r0 + (t + 1) * 128, si * q:(si + 1) * q])
                    else:
                        nc.sync.dma_start(s[:], wblob[r0 + t * 128:r0 + (t + 1) * 128, :])
                    whh_slabs[k][t] = s

            def emit_wih_dma(k):
                if ablate_weight_dma:
                    return
                r0 = (k - 1) * WROWS_PER_LAYER
                for t in range(8):
                    s = wpool.tile([128, 2 * O], BF16, name=f"wih{k}_{t}",
                                   tag="wslab")
                    if split_slab:
                        q = 2 * O // split_slab
                        for si in range(split_slab):
                            nc.sync.dma_start(
                                s[:, si * q:(si + 1) * q],
                                wblob[r0 + t * 128:r0 + (t + 1) * 128, si * q:(si + 1) * q])
                    else:
                        nc.sync.dma_start(s[:], wblob[r0 + t * 128:r0 + (t + 1) * 128, :])
                    wih_slabs[k][t] = s

            # consumption order: w1 i1 w2 w3 i2 w4 i3 w5 i4 i5
            emit_whh_dma(1); emit_wih_dma(1)
            emit_whh_dma(2); emit_whh_dma(3)
            emit_wih_dma(2)
            emit_whh_dma(4); emit_wih_dma(3)
            emit_whh_dma(5); emit_wih_dma(4)
            emit_wih_dma(5)

            # ---------- phase C: hi/lo splits of stationary vectors (DVE) ----------
            def split(src, F, nm):
                hi = small.tile([128, F], BF16, name=f"{nm}_hi")
                nc.vector.tensor_copy(hi[:], src[:])
                hi32 = small.tile([128, F], F32, name=f"{nm}_hi32")
                nc.vector.tensor_copy(hi32[:], hi[:])
                res = small.tile([128, F], F32, name=f"{nm}_res")
                nc.vector.tensor_sub(res[:], src[:], hi32[:])
                lo = small.tile([128, F], BF16, name=f"{nm}_lo")
                nc.vector.tensor_copy(lo[:], res[:])
                return hi, lo

            if two_pass:
                hsplit = {}
                xsplit = {}
                xstat = {1: x1_sb}
            else:
                hsplit = {k: split(h_sb[k], 16, f"h{k}") for k in range(1, 6)}
                xsplit = {1: split(x1_sb, 8, "x1")}
                xstat = {}

            # ---------- phase D: layered compute ----------
            psum_g = {}
            agin = {k: dpool.tile([1, O], F32, name=f"agin{k}") for k in range(1, 6)}
            agout = {k: dpool.tile([128, 64], F32, name=f"agout{k}") for k in range(1, 6)}
            ag2in = {k: dpool.tile([128, 1], F32, name=f"ag2in{k}") for k in range(1, 6)}
            ag2out = {k: dpool.tile([8, 128], F32, name=f"ag2out{k}") for k in range(1, 6)}
            y_cm = {}
            h_new = {}
            replica = [list(range(num_devices))]

            def emit_gates_whh(k):
                """bias + Whh@h part of layer k's gate pre-activations (PE)."""
                pg = psg.tile([1, O], F32, name=f"psg{k}", tag="psg")
                psum_g[k] = pg
                bias_stop = ablate_gate_mms or ablate_weight_dma
                for n0 in (0, 512):
                    nc.tensor.matmul(pg[0:1, n0:n0 + 512], one_sb[:],
                                     b_sb[k][0:1, n0:n0 + 512],
                                     start=True, stop=bias_stop)
                if ablate_gate_mms or ablate_weight_dma:
                    return
                if two_pass:
                    passes = ((h_sb[k], 0), (h_sb[k], O))
                else:
                    hh, hl = hsplit[k]
                    passes = ((hh, 0), (hh, O), (hl, 0))
                for t in range(16):
                    s = whh_slabs[k][t]
                    for lh, base in passes:
                        for n0 in (0, 512):
                            nc.tensor.matmul(pg[0:1, n0:n0 + 512],
                                             lh[:, t:t + 1],
                                             s[:, base + n0:base + n0 + 512],
                                             start=False, stop=False)

            def emit_gates_wih(k):
                """Wih@x part (PE) + psum->DRAM + AllGather of gate slice."""
                pg = psum_g[k]
                if two_pass:
                    xs = xstat[k]
                    passes = ((xs, 0), (xs, O))
                else:
                    xh, xl = xsplit[k]
                    passes = ((xh, 0), (xh, O), (xl, 0))
                if not (ablate_gate_mms or ablate_weight_dma):
                    for t in range(8):
                        s = wih_slabs[k][t]
                        for pi, (lh, base) in enumerate(passes):
                            last = (t == 7 and pi == len(passes) - 1)
                            for n0 in (0, 512):
                                nc.tensor.matmul(pg[0:1, n0:n0 + 512],
                                                 lh[:, t:t + 1],
                                                 s[:, base + n0:base + n0 + 512],
                                                 start=False, stop=last)
                gsb = work.tile([1, O], F32, name=f"gsb{k}", tag="gsb")
                nc.vector.tensor_copy(gsb[0:1, 0:512], pg[0:1, 0:512])
                nc.scalar.copy(gsb[0:1, 512:O], pg[0:1, 512:O])
                nc.scalar.dma_start(agin[k][:], gsb[:])
                if local_ag:
                    nc.scalar.dma_start(
                        agout[k].rearrange("p j -> (p j)")[0:O], agin[k][0, :])
                else:
                    nc.gpsimd.collective_compute(
                        "AllGather", mybir.AluOpType.bypass,
                        replica_groups=replica,
                        ins=[agin[k].opt()], outs=[agout[k].opt()])

            def emit_elem(k):
                """Gate nonlinearities + c/h update (ACT + DVE), store hn/cn."""
                gates = work.tile([128, 64], F32, name=f"gates{k}", tag="gates")
                nc.scalar.dma_start(gates[:], agout[k][:])
                si = work.tile([128, 16], F32, name=f"si{k}", tag="si")
                sf = work.tile([128, 16], F32, name=f"sf{k}", tag="sf")
                tg = work.tile([128, 16], F32, name=f"tg{k}", tag="tg")
                so = work.tile([128, 16], F32, name=f"so{k}", tag="so")
                nc.scalar.activation(si[:], gates[:, 0:16], AF.Sigmoid)
                nc.scalar.activation(sf[:], gates[:, 16:32], AF.Sigmoid)
                nc.scalar.activation(tg[:], gates[:, 32:48], AF.Tanh)
                nc.scalar.activation(so[:], gates[:, 48:64], AF.Sigmoid)
                t1 = work.tile([128, 16], F32, name=f"t1_{k}", tag="t1")
                t2 = work.tile([128, 16], F32, name=f"t2_{k}", tag="t2")
                cn = work.tile([128, 16], F32, name=f"cn{k}", tag="cnew", bufs=5)
                tcn = work.tile([128, 16], F32, name=f"tc{k}", tag="tcn")
                hn = work.tile([128, 16], F32, name=f"hn{k}", tag="hnew", bufs=5)
                nc.vector.tensor_mul(t1[:], sf[:], c_sb[k][:])
                nc.vector.tensor_mul(t2[:], si[:], tg[:])
                nc.vector.tensor_add(cn[:], t1[:], t2[:])
                nc.scalar.activation(tcn[:], cn[:], AF.Tanh)
                nc.vector.tensor_mul(hn[:], so[:], tcn[:])
                h_new[k] = hn
                nc.scalar.dma_start(
                    osl(f"hn{k}").rearrange("(p j) -> p j", p=128), hn[:])
                nc.scalar.dma_start(
                    osl(f"cn{k}").rearrange("(p j) -> p j", p=128), cn[:])

            def emit_head(k):
                """Sharded head: y[c*128:(c+1)*128] = Wout_slice @ hn + bout_slice."""
                py = psy.tile([128, 1], F32, name=f"psy{k}", tag="psy")
                for t in range(16):
                    nc.tensor.matmul(py[:], wout_sb[:, t * 128:(t + 1) * 128],
                                     h_new[k][:, t:t + 1],
                                     start=(t == 0), stop=(t == 15))
                ysb = work.tile([128, 1], F32, name=f"ysb{k}", tag="ysb")
                nc.vector.tensor_add(ysb[:], py[:], boutc_sb[:])
                nc.scalar.dma_start(ag2in[k][:], ysb[:])
                if local_ag:
                    nc.scalar.dma_start(
                        ag2out[k].rearrange("m p -> (m p)")[0:128],
                        ag2in[k][:, 0])
                else:
                    nc.gpsimd.collective_compute(
                        "AllGather", mybir.AluOpType.bypass,
                        replica_groups=replica,
                        ins=[ag2in[k].opt()], outs=[ag2out[k].opt()])

            def emit_redist(k):
                """Full y back to column-major (128, 8) via PE transpose."""
                yrow = work.tile([8, 128], F32, name=f"yrow{k}", tag="yrow")
                nc.scalar.dma_start(yrow[:], ag2out[k][:])
                pt = pst.tile([128, 8], F32, name=f"pst{k}", tag="pst")
                nc.tensor.transpose(pt[:], yrow[:], id8_sb[:])
                yc = small.tile([128, 8], F32, name=f"ycm{k}")
                nc.vector.tensor_copy(yc[:], pt[:])
                y_cm[k] = yc
                if k < 5:
                    if k == 1:
                        xn = yc
                    else:
                        xn = small.tile([128, 8], F32, name=f"x{k+1}cm")
                        nc.vector.tensor_add(xn[:], yc[:], y_cm[k - 1][:])
                    if two_pass:
                        xstat[k + 1] = xn
                    else:
                        xsplit[k + 1] = split(xn, 8, f"x{k+1}")

            def emit_softmax():
                e = small.tile([128, 8], F32, name="esm")
                nc.scalar.activation(e[:], y_cm[5][:], AF.Exp)
                s = small.tile([128, 1], F32, name="ssm")
                nc.vector.reduce_sum(s[:], e[:], axis=mybir.AxisListType.X)
                tot = psy.tile([128, 1], F32, name="totps", tag="psy")
                nc.tensor.matmul(tot[:], ones2d_sb[:], s[:], start=True, stop=True)
                inv = small.tile([128, 1], F32, name="invsm")
                nc.vector.reciprocal(inv[:], tot[:])
                osb = small.tile([128, 8], F32, name="osb")
                nc.vector.tensor_scalar_mul(osb[:], e[:], inv[:])
                nc.scalar.dma_start(
                    osl("outp").rearrange("(p m) -> p m", p=128), osb[:])

            # Whh lookahead keeps PE fed during collective waits
            emit_gates_whh(1)
            emit_gates_wih(1)
            emitted_whh = 1
            for k in range(1, 6):
                while emitted_whh < min(5, k + lookahead - 1):
                    emitted_whh += 1
                    emit_gates_whh(emitted_whh)
                emit_elem(k)
                emit_head(k)
                if emitted_whh < min(5, k + lookahead):
                    emitted_whh += 1
                    emit_gates_whh(emitted_whh)
                emit_redist(k)
                if k + 1 <= 5:
                    emit_gates_wih(k + 1)
            emit_softmax()

    nc.compile()
    return nc


# --------------------------------------------------------------------------
# Host-side input prep (sharding + layout + blob packing)
# --------------------------------------------------------------------------

def prep_in_maps(inputs):
    inp = {k: np.asarray(v, dtype=np.float32) for k, v in inputs.items()}
    x1 = inp["input"].reshape(O)
    WoutT = np.ascontiguousarray(inp["Wout"].T)          # (H, O)
    bout = inp["bout"].reshape(O)

    def pack(wt):
        hi = wt.astype(ml_dtypes.bfloat16)
        lo = (wt - hi.astype(np.float32)).astype(ml_dtypes.bfloat16)
        return np.concatenate([hi, lo], axis=1)

    in_maps = []
    for c in range(NC):
        pp, jj = np.meshgrid(np.arange(16), np.arange(64), indexing="ij")
        idx = (jj * 128 + 16 * c + pp).reshape(-1)       # slice row s -> gate row

        wrows = []
        for k in range(1, 6):
            wrows.append(pack(inp[f"Wih{k}"][idx, :].T))     # (O, 2O)
            wrows.append(pack(inp[f"Whh{k}"][idx, :].T))     # (H, 2O)
        wblob = np.ascontiguousarray(np.concatenate(wrows, axis=0))

        v = np.zeros(VLEN, np.float32)

        def put(name, arr):
            off, n = VOFF[name]
            a = np.asarray(arr, np.float32).ravel()
            assert a.size == n, (name, a.size, n)
            v[off:off + n] = a

        put("x1cm", x1)                       # flat natural; device rearranges
        put("ident8", np.eye(8, dtype=np.float32))
        put("one", [1.0])
        put("ones2d", np.ones(128 * 128, np.float32))
        put("boutc", bout[c * 128:(c + 1) * 128])
        put("woutts", WoutT[:, c * 128:(c + 1) * 128].reshape(16, 128, 128))
        for k in range(1, 6):
            put(f"b{k}", (inp[f"bih{k}"] + inp[f"bhh{k}"])[idx])
            put(f"hcm{k}", inp[f"h{k}"].reshape(H))
            put(f"ccm{k}", inp[f"c{k}"].reshape(H))
        in_maps.append({"wblob": wblob, "vblob": v})
    return in_maps


# --------------------------------------------------------------------------
# Cached PJRT runner (the axon redirect path of bass_utils.run_bass_kernel_spmd,
# with the jitted executable cached so repeated kernel() calls don't recompile)
# --------------------------------------------------------------------------

_RT = None


def _make_runner(nc):
    import jax
    from jax.sharding import Mesh, PartitionSpec
    from jax.experimental.shard_map import shard_map
    from concourse import bass2jax
    bass2jax.install_neuronx_cc_hook()

    partition_name = (nc.partition_id_tensor.name
                      if nc.partition_id_tensor is not None else None)
    in_names, out_names, out_avals, zero_shapes = [], [], [], []
    for alloc in nc.m.functions[0].allocations:
        if not isinstance(alloc, mybir.MemoryLocationSet):
            continue
        name = alloc.memorylocations[0].name
        if alloc.kind == "ExternalInput":
            if name != partition_name:
                in_names.append(name)
        elif alloc.kind == "ExternalOutput":
            out_names.append(name)
            shape = tuple(alloc.tensor_shape)
            dtype = mybir.dt.np(alloc.dtype)
            out_avals.append(jax.core.ShapedArray(shape, dtype))
            zero_shapes.append((shape, dtype))
    n_params = len(in_names)
    n_outs = len(out_names)
    all_in_names = tuple(in_names) + tuple(out_names)
    if partition_name is not None:
        all_in_names = all_in_names + (partition_name,)

    def _body(*args):
        operands = list(args)
        if partition_name is not None:
            operands.append(bass2jax.partition_id_tensor())
        outs = bass2jax._bass_exec_p.bind(
            *operands,
            out_avals=tuple(out_avals),
            in_names=all_in_names,
            out_names=tuple(out_names),
            lowering_input_output_aliases=(),
            sim_require_finite=True,
            sim_require_nnan=True,
            nc=nc,
        )
        return tuple(outs)

    devices = jax.devices()[:NC]
    mesh = Mesh(np.asarray(devices), ("core",))
    sharded = jax.jit(
        shard_map(_body, mesh=mesh,
                  in_specs=(PartitionSpec("core"),) * (n_params + n_outs),
                  out_specs=(PartitionSpec("core"),) * n_outs,
                  check_rep=False),
        donate_argnums=tuple(range(n_params, n_params + n_outs)),
        keep_unused=True,
    )
    return sharded, in_names, out_names, out_avals, zero_shapes


def get_runtime():
    global _RT
    if _RT is None:
        nc = build_nc()
        _RT = _make_runner(nc)
    return _RT


def run_in_maps(in_maps):
    """Run the SPMD program; returns core 0's output map."""
    sharded, in_names, out_names, out_avals, zero_shapes = get_runtime()
    concat_in = [np.concatenate([in_maps[c][nm] for c in range(NC)], axis=0)
                 for nm in in_names]
    concat_zeros = [np.zeros((NC * s[0], *s[1:]), dt) for s, dt in zero_shapes]
    outs = sharded(*concat_in, *concat_zeros)
    return {nm: np.asarray(outs[i]).reshape(NC, *out_avals[i].shape)[0]
            for i, nm in enumerate(out_names)}


# --------------------------------------------------------------------------
# Public entry point
# --------------------------------------------------------------------------

def kernel(**inputs):
    res0 = run_in_maps(prep_in_maps(inputs))
    ob = res0["oblob"]

    def get(name, j):
        # stored [p, j] row-major; natural vector order is j*128+p
        off, n = OOFF[name]
        return np.ascontiguousarray(ob[off:off + n].reshape(128, j).T).ravel()

    out = get("outp", 8).reshape(1, O).astype(np.float32)
    ret = [out]
    for k in range(1, 6):
        hn = get(f"hn{k}", 16).reshape(1, 1, H).astype(np.float32)
        cn = get(f"cn{k}", 16).reshape(1, 1, H).astype(np.float32)
        ret += [hn, cn]
    return tuple(ret)
